# revision 5
# baseline (speedup 1.0000x reference)
"""Multi-head self-attention Trainium2 kernel (8 NeuronCores).

Problem: B=4, N=2048, D=1024, H=16 heads of dim 64, fp32 in/out.

Sharding: 8 cores = 4 batches x 2 head-groups. Core c handles batch c//2
and heads (c%2)*8 .. (c%2)*8+7 (a 512-wide slice of the hidden dim).
Each core computes q/k/v projections for its head slice, attention for
its 8 heads, and a partial out-projection (contraction over its 512
attention dims). Host sums the two partials per batch.

Device dataflow (per core), all matmuls bf16 with fp32 PSUM accumulate:
  - x^T (host-pretransposed, bf16) lives in SBUF as 8 [128, 2048] tiles.
  - q_a/k_a = W^T.T @ x^T in "layout a" [head_dim-part, token-free].
  - v in "layout b" [token-part, head_dim-free], restrided into per-head
    65-column segments whose last column is ones (gives the softmax
    denominator for free during the PV matmul).
  - scores computed transposed: S^T[j, i] = k_a^T q_a (K=64 contraction),
    exp on ScalarE (scale=1/8 folded in, no max subtraction -- scores are
    ~N(0,1) so exp is safe), output P^T bf16 straight to SBUF.
  - PV: out[65, i] += v'[j,:65]^T P^T[j, i]; row 64 = sum_j exp = denom.
  - normalize: reciprocal of row 64, partition-broadcast, multiply.
  - out-projection: o[token, d_out] = attn^T.T @ Wo_slice^T, fp32 out.

Biases: bq applied on device (per-partition in layout a). bk cancels
exactly in softmax (adds a per-query constant to scores). bv and bo are
folded on host: attn rows sum to 1 so bv passes through linearly.
"""

import numpy as np
import ml_dtypes

BF16 = ml_dtypes.bfloat16

HIDDEN = 1024
N_TOK = 2048
BATCH = 4
N_CORES = 8

_CACHE = {}


def _build_nc(D, N):
    """Build + compile the per-core Bass program.

    Per-core tensor shapes (DL = D // 2 local q/k/v width):
      xT  [D, N]  bf16   : x[b] transposed
      wqT/wkT/wvT [D, DL] bf16 : W[hs:hs+DL, :].T
      woT [DL, D] bf16   : Wo[:, hs:hs+DL].T
      bqt [128, DL//128] f32 : bq slice, chunked per partition
      o   [N, D]  f32    : partial output (host sums pairs)
    """
    import concourse.bacc as bacc
    import concourse.mybir as mybir
    import concourse.tile as tile
    from contextlib import ExitStack

    dt = mybir.dt
    P = 128
    DL = D // 2
    KC = D // P          # d_model chunks (8)
    MC = DL // P         # head-dim chunks == head pairs (4)
    NHL = DL // 64       # local heads (8)
    NT = N // P          # token tiles (16)
    ICB = N // 2         # i-block width (1024)
    MMW = min(512, ICB)  # matmul moving width
    NSL = ICB // MMW     # moving slices per i-block (2)
    JT = NT              # j tiles (16)
    PW = min(512, N)     # projection moving width
    NPC = N // PW        # projection n-chunks (4)

    nc = bacc.Bacc("TRN2", target_bir_lowering=False, debug=False)

    xT = nc.dram_tensor("xT", [D, N], dt.bfloat16, kind="ExternalInput")
    wqT = nc.dram_tensor("wqT", [D, DL], dt.bfloat16, kind="ExternalInput")
    wkT = nc.dram_tensor("wkT", [D, DL], dt.bfloat16, kind="ExternalInput")
    wvT = nc.dram_tensor("wvT", [D, DL], dt.bfloat16, kind="ExternalInput")
    woT = nc.dram_tensor("woT", [DL, D], dt.bfloat16, kind="ExternalInput")
    bqt = nc.dram_tensor("bqt", [P, MC], dt.float32, kind="ExternalInput")
    o = nc.dram_tensor("o", [N, D], dt.float32, kind="ExternalOutput")

    with tile.TileContext(nc) as tc, ExitStack() as ctx:
        pers = ctx.enter_context(tc.tile_pool(name="pers", bufs=1))
        work = ctx.enter_context(tc.tile_pool(name="work", bufs=2))
        pmm = ctx.enter_context(tc.tile_pool(name="pmm", bufs=2, space="PSUM"))
        ppv = ctx.enter_context(tc.tile_pool(name="ppv", bufs=2, space="PSUM"))

        # ---- persistent SBUF tiles + input DMAs ----
        xt_t = [pers.tile([P, N], dt.bfloat16, name=f"xT{k}", tag=f"xT{k}") for k in range(KC)]
        wq_t = [pers.tile([P, DL], dt.bfloat16, name=f"wq{k}", tag=f"wq{k}") for k in range(KC)]
        wk_t = [pers.tile([P, DL], dt.bfloat16, name=f"wk{k}", tag=f"wk{k}") for k in range(KC)]
        wv_t = [pers.tile([P, DL], dt.bfloat16, name=f"wv{k}", tag=f"wv{k}") for k in range(KC)]
        wo_t = [pers.tile([P, D], dt.bfloat16, name=f"wo{m}", tag=f"wo{m}") for m in range(MC)]
        bq_t = pers.tile([P, MC], dt.float32, name="bqt_sb", tag="bqt")
        qa = [pers.tile([P, N], dt.bfloat16, name=f"qa{m}", tag=f"qa{m}") for m in range(MC)]
        ka = [pers.tile([P, N], dt.bfloat16, name=f"ka{m}", tag=f"ka{m}") for m in range(MC)]
        vp = [pers.tile([P, NHL * 65], dt.bfloat16, name=f"vp{t}", tag=f"vp{t}") for t in range(NT)]
        attn = [pers.tile([P, N], dt.bfloat16, name=f"attn{m}", tag=f"attn{m}") for m in range(MC)]

        for k in range(KC):
            nc.sync.dma_start(wk_t[k][:], wkT[k * P:(k + 1) * P, :])
            nc.sync.dma_start(wv_t[k][:], wvT[k * P:(k + 1) * P, :])
            nc.sync.dma_start(wq_t[k][:], wqT[k * P:(k + 1) * P, :])
            nc.sync.dma_start(xt_t[k][:], xT[k * P:(k + 1) * P, :])
        for m in range(MC):
            nc.sync.dma_start(wo_t[m][:], woT[m * P:(m + 1) * P, :])
        nc.sync.dma_start(bq_t[:], bqt[:, :])

        # ---- v projection: out [token, DL], restrided into 65-col segments
        for t in range(NT):
            ps = pmm.tile([P, DL], dt.float32, tag="mm")
            for k in range(KC):
                for s in range(0, DL, 512):
                    w = min(512, DL - s)
                    nc.tensor.matmul(
                        out=ps[:, s:s + w],
                        lhsT=xt_t[k][:, t * P:(t + 1) * P],
                        rhs=wv_t[k][:, s:s + w],
                        start=(k == 0),
                        stop=(k == KC - 1),
                    )
            seg = vp[t][:].rearrange("p (s c) -> p s c", c=65)
            nc.vector.memset(seg[:, :, 64:65], 1.0)
            nc.vector.tensor_copy(
                seg[:, :, 0:64],
                ps[:].rearrange("p (s c) -> p s c", c=64),
            )

        # ---- k and q projections, per head pair m: layout a [hd, token]
        for m in range(MC):
            for (wt, bias, dst) in ((wk_t, None, ka), (wq_t, bq_t, qa)):
                for n2 in range(0, N, ICB):
                    ps = pmm.tile([P, ICB], dt.float32, tag="mm")
                    for k in range(KC):
                        for s in range(0, ICB, MMW):
                            nc.tensor.matmul(
                                out=ps[:, s:s + MMW],
                                lhsT=wt[k][:, m * P:(m + 1) * P],
                                rhs=xt_t[k][:, n2 + s:n2 + s + MMW],
                                start=(k == 0),
                                stop=(k == KC - 1),
                            )
                    if bias is None:
                        nc.vector.tensor_copy(
                            dst[m][:, n2:n2 + ICB], ps[:])
                    else:
                        nc.vector.tensor_scalar_add(
                            dst[m][:, n2:n2 + ICB], ps[:], bias[:, m:m + 1])

        # ---- attention + out-projection, i-block outer ----
        for ib in range(2):
            i0 = ib * ICB
            for h in range(NHL):
                m = h // 2
                r = (h % 2) * 64
                pv = ppv.tile([65, ICB], dt.float32, tag="pv")
                for j in range(JT):
                    ps = pmm.tile([P, ICB], dt.float32, tag="mm")
                    for s in range(0, ICB, MMW):
                        nc.tensor.matmul(
                            out=ps[:, s:s + MMW],
                            lhsT=ka[m][r:r + 64, j * P:(j + 1) * P],
                            rhs=qa[m][r:r + 64, i0 + s:i0 + s + MMW],
                            start=True,
                            stop=True,
                        )
                    pt = work.tile([P, ICB], dt.bfloat16, tag="pt", bufs=JT + 2)
                    nc.scalar.activation(
                        pt[:], ps[:],
                        mybir.ActivationFunctionType.Exp,
                        bias=0.0, scale=0.125,
                    )
                    for s in range(0, ICB, MMW):
                        nc.tensor.matmul(
                            out=pv[:, s:s + MMW],
                            lhsT=vp[j][:, h * 65:(h + 1) * 65],
                            rhs=pt[:, s:s + MMW],
                            start=(j == 0),
                            stop=(j == JT - 1),
                        )
                recip = work.tile([1, ICB], dt.float32, tag="recip")
                nc.vector.reciprocal(recip[:], pv[64:65, :])
                bcast = work.tile([64, ICB], dt.float32, tag="bcast")
                nc.gpsimd.partition_broadcast(bcast[:], recip[:])
                nc.vector.tensor_tensor(
                    attn[m][r:r + 64, i0:i0 + ICB],
                    pv[0:64, :],
                    bcast[:],
                    mybir.AluOpType.mult,
                )
            # out-projection for this i-block's tokens
            for t in range(ib * (NT // 2), (ib + 1) * (NT // 2)):
                ps = pmm.tile([P, D], dt.float32, tag="mm")
                for k in range(MC):
                    for s in range(0, D, 512):
                        w = min(512, D - s)
                        nc.tensor.matmul(
                            out=ps[:, s:s + w],
                            lhsT=attn[k][:, t * P:(t + 1) * P],
                            rhs=wo_t[k][:, s:s + w],
                            start=(k == 0),
                            stop=(k == MC - 1),
                        )
                oe = work.tile([P, D], dt.float32, tag="oev")
                nc.vector.tensor_copy(oe[:], ps[:])
                nc.sync.dma_start(o[t * P:(t + 1) * P, :], oe[:])

    nc.compile()
    return nc


def _get_nc(D, N):
    key = (D, N)
    if key not in _CACHE:
        _CACHE[key] = _build_nc(D, N)
    return _CACHE[key]


def _make_in_maps(x, Wq, bq, Wk, Wv, Wo, D, N):
    DL = D // 2
    MC = DL // 128
    in_maps = []
    for c in range(N_CORES):
        b = c // 2
        hs = (c % 2) * DL
        in_maps.append({
            "xT": np.ascontiguousarray(x[b].T).astype(BF16),
            "wqT": np.ascontiguousarray(Wq[hs:hs + DL, :].T).astype(BF16),
            "wkT": np.ascontiguousarray(Wk[hs:hs + DL, :].T).astype(BF16),
            "wvT": np.ascontiguousarray(Wv[hs:hs + DL, :].T).astype(BF16),
            "woT": np.ascontiguousarray(Wo[:, hs:hs + DL].T).astype(BF16),
            "bqt": np.ascontiguousarray(
                bq[hs:hs + DL].reshape(MC, 128).T).astype(np.float32),
        })
    return in_maps


def _run(x, Wq, bq, Wk, bk, Wv, bv, Wo, bo, trace=False):
    from concourse.bass_utils import run_bass_kernel_spmd

    x = np.asarray(x, np.float32)
    B, N, D = x.shape
    nc = _get_nc(D, N)
    in_maps = _make_in_maps(
        x, np.asarray(Wq, np.float32), np.asarray(bq, np.float32),
        np.asarray(Wk, np.float32), np.asarray(Wv, np.float32),
        np.asarray(Wo, np.float32), D, N)
    res = run_bass_kernel_spmd(
        nc, in_maps, list(range(N_CORES)), trace=trace)

    bv = np.asarray(bv, np.float32)
    bo = np.asarray(bo, np.float32)
    extra = bv @ np.asarray(Wo, np.float32).T + bo  # exact linear fold
    out = np.empty((B, N, D), np.float32)
    for b in range(B):
        out[b] = res.results[2 * b]["o"] + res.results[2 * b + 1]["o"] + extra
    return out, res


def kernel(x, Wq, bq, Wk, bk, Wv, bv, Wo, bo):
    out, _ = _run(x, Wq, bq, Wk, bk, Wv, bv, Wo, bo, trace=False)
    return out
